# revision 25
# baseline (speedup 1.0000x reference)
"""Bidirectional cross-attention kernel for Trainium2, SPMD over 8 NeuronCores.

Reference (per batch b, heads K=8, head dim D=32, N=128*128 pixels):
    q   = softmax_d(Wq @ x)
    for branch j in {1,2}:
        key   = softmax_n(Wk_j @ ref_j)          # softmax over the pixel dim
        v     = Wv_j @ ref_j
        ctx_j = key @ v^T                        # [K,D,D]
        out_j = per-pixel  q @ ctx_j^T
    y = Wo @ concat(out_1, out_2)

Sharding: 8 cores = batch(4) x head-group(2).  Each core owns 4 of the 8
heads for its batch: projections, softmaxes, ctx and the out einsum are
fully head-local; the final Wo projection is computed as a partial sum
over the core's 256 (of 512) concat channels, and the host adds the two
partial outputs per batch.  No cross-core communication on device.

Numerics: bf16 matmul inputs (host-cast), fp32 PSUM accumulation, fp32
scalar/vector math.  Softmaxes skip max-subtraction (logits ~N(0,1), exp
is safe in fp32).

SBUF layout: tensors with >128 channels are stored as [128, k*cols] with
128-channel k-tiles side by side in the free dim.  Key/value tensors are
kept in transposed [pixel, channel] layout (needed for the ctx einsum,
whose contraction runs over pixels).
"""

import numpy as np
import ml_dtypes

import concourse.bass as bass
import concourse.bacc as bacc
import concourse.tile as tile
from concourse import mybir
from concourse.bass_utils import run_bass_kernel_spmd

BF16 = mybir.dt.bfloat16
F32 = mybir.dt.float32
AF = mybir.ActivationFunctionType

B, C, H, W = 4, 256, 128, 128
K, D = 8, 32
N = H * W
N_CORES = 8


def build_nc(n_loc=N):
    nc = bacc.Bacc("TRN2", target_bir_lowering=False, debug=False,
                   num_devices=N_CORES)

    nt = n_loc // 128        # 128-pixel tiles (128)
    nt512 = n_loc // 512     # 512-pixel tiles (32)

    # ---- I/O (weights pre-transposed, head-group-sliced, k-tiled on host) --
    x = nc.declare_dram_parameter("x", [C, n_loc], BF16, isOutput=False)
    r1 = nc.declare_dram_parameter("r1", [C, n_loc], BF16, isOutput=False)
    r2 = nc.declare_dram_parameter("r2", [C, n_loc], BF16, isOutput=False)
    # wq: [128, 2*128]  col chunk 128k = Wq.T[128k:128k+128, our 128 channels]
    wq = nc.declare_dram_parameter("wq", [128, 2 * 128], BF16, isOutput=False)
    # wkv_j: [128, 2*256] col chunk 256k = [WkT | WvT](our heads)[128k:, :]
    wkv1 = nc.declare_dram_parameter("wkv1", [128, 2 * 256], BF16, isOutput=False)
    wkv2 = nc.declare_dram_parameter("wkv2", [128, 2 * 256], BF16, isOutput=False)
    # wo: [128, 2*256]  col chunk 256k = Wo.T[our 256 concat channels][128k:, :]
    wo = nc.declare_dram_parameter("wo", [128, 2 * 256], BF16, isOutput=False)
    ones4 = nc.declare_dram_parameter("ones4", [128, 32], BF16, isOutput=False)
    ones4T = nc.declare_dram_parameter("ones4T", [128, 128], BF16, isOutput=False)

    y = nc.declare_dram_parameter("y", [C, n_loc], BF16, isOutput=True)

    refs = [r1, r2]

    with tile.TileContext(nc) as tc:
        with (
            tc.tile_pool(name="weights", bufs=1) as wpool,
            tc.tile_pool(name="persist", bufs=1) as ppool,
            tc.tile_pool(name="io", bufs=3) as iopool,
            tc.tile_pool(name="work", bufs=3) as wkpool,
        ):
            # ---- weights / constants (wkv1 first on sync so branch-0 can
            # start; everything else rides the gpsimd queue) ----
            wkv_t = []
            for j, wsrc in enumerate((wkv1, wkv2)):
                t = wpool.tile([128, 2 * 256], BF16, tag=f"wkv{j}", name=f"wkv_t{j}")
                (nc.sync if j == 0 else nc.gpsimd).dma_start(t[:], wsrc[:, :])
                wkv_t.append(t)
            wq_t = wpool.tile([128, 2 * 128], BF16, tag="wq")
            nc.gpsimd.dma_start(wq_t[:], wq[:, :])
            ones4_t = wpool.tile([128, 32], BF16, tag="o4")
            nc.gpsimd.dma_start(ones4_t[:], ones4[:, :])
            ones4T_t = wpool.tile([128, 128], BF16, tag="o4T")
            nc.gpsimd.dma_start(ones4T_t[:], ones4T[:, :])
            wo_t = wpool.tile([128, 2 * 256], BF16, tag="wo")
            nc.gpsimd.dma_start(wo_t[:], wo[:, :])

            compact = ppool.tile([128, 64], F32, tag="compact")
            compact_bf = ppool.tile([128, 64], BF16, tag="compact_bf")
            mt_sb = ppool.tile([128, 256], BF16, tag="mt_sb")
            recipT = ppool.tile([128, 2], F32, tag="recipT")
            expq = ppool.tile([128, n_loc], BF16, tag="expq")
            nzc = (nt512 + 3) // 4
            zqr = ppool.tile([128, 512 * nzc], BF16, tag="zqr")

            CH = 4               # kv: 128-pixel tiles per chunk
            nch = nt // CH       # 32 chunks per branch

            with (
                tc.tile_pool(name="kvstage", bufs=1) as kvpool,
                tc.tile_pool(name="psA", bufs=2, space="PSUM") as psA,
                tc.tile_pool(name="psAcc", bufs=1, space="PSUM") as psAcc,
                tc.tile_pool(name="psM", bufs=1, space="PSUM") as psM,
                tc.tile_pool(name="psQ", bufs=2, space="PSUM") as psQ,
            ):
                ekt_all = kvpool.tile([128, nt * 128], BF16, tag="ekt_all")
                vt_all = kvpool.tile([128, nt * 128], BF16, tag="vt_all")
                ones1_t = wpool.tile([128, 1], BF16, tag="o1")
                nc.vector.memset(ones1_t[:], 1.0)
                # one PSUM bank: cols 128j..128j+128 = branch j ctx[c,d];
                # col 256+j = zkT_j = sum_n exp(k_j[c,n]) on partitions c
                ctx_ps = psAcc.tile([128, 258], F32, tag="ctx")
                # MT[d, o] = sum_j sum_c ctx_j[c,d]/zk_j[c] * WoT[c_j, o]
                mt_ps = psM.tile([128, 256], F32, tag="mt")

                def pass1(j, ch):
                    base = ch * CH * 128
                    r_t = iopool.tile([128, CH * 256], BF16, tag="rchunk",
                                      name=f"r_{j}_{ch}")
                    for k in range(2):
                        dma_eng = nc.sync if (ch + k) % 2 == 0 else nc.gpsimd
                        dma_eng.dma_start(
                            r_t[:, CH * 128 * k:CH * 128 * (k + 1)],
                            refs[j][128 * k:128 * (k + 1), base:base + CH * 128])
                    kv_ps = psA.tile([128, CH * 256], F32, tag="kv",
                                     name=f"kv_{j}_{ch}")
                    for t in range(CH):
                        for k in range(2):
                            nc.tensor.matmul(
                                kv_ps[:, 256 * t:256 * (t + 1)],
                                r_t[:, CH * 128 * k + 128 * t:
                                       CH * 128 * k + 128 * (t + 1)],
                                wkv_t[j][:, 256 * k:256 * (k + 1)],
                                start=(k == 0), stop=(k == 1),
                            )
                    ek_sl = ekt_all[:, ch * CH * 128:(ch + 1) * CH * 128]
                    nc.scalar.activation(
                        ek_sl.rearrange("p (t c) -> p t c", t=CH),
                        kv_ps[:].rearrange("p (t c) -> p t c", t=CH)[:, :, 0:128],
                        AF.Exp,
                    )
                    vt_sl = vt_all[:, ch * CH * 128:(ch + 1) * CH * 128]
                    nc.vector.tensor_copy(
                        vt_sl.rearrange("p (t c) -> p t c", t=CH),
                        kv_ps[:].rearrange("p (t c) -> p t c", t=CH)[:, :, 128:256],
                    )

                def pass2(j, ch):
                    # ctx[c,d] accumulation; the N=1 ones-matmul reuses the
                    # same ekt stationary to accumulate zkT[c] nearly free.
                    for t in range(ch * CH, (ch + 1) * CH):
                        nc.tensor.matmul(
                            ctx_ps[:, 128 * j:128 * (j + 1)],
                            ekt_all[:, 128 * t:128 * (t + 1)],
                            vt_all[:, 128 * t:128 * (t + 1)],
                            start=(t == 0), stop=(t == nt - 1),
                        )
                        nc.tensor.matmul(
                            ctx_ps[:, 256 + j:257 + j],
                            ekt_all[:, 128 * t:128 * (t + 1)],
                            ones1_t[:],
                            start=(t == 0), stop=(t == nt - 1),
                        )

                def compact_j(j):
                    # fold 1/zk[c] into the diag 32x32 blocks of ctx[c,d]
                    # (c on partitions), cast to bf16, then fold the scaled
                    # ctx into Wo:  mt[d, o] += sum_c ctx_j[c,d] WoT_j[c, o]
                    # head-block-diagonal, so 4 PE-packed matmuls per branch.
                    nc.vector.reciprocal_approx_fast(
                        recipT[:, j:j + 1], ctx_ps[:, 256 + j:257 + j])
                    for a in range(4):
                        nc.vector.tensor_scalar_mul(
                            compact[32 * a:32 * (a + 1), 32 * j:32 * j + 32],
                            ctx_ps[32 * a:32 * (a + 1),
                                   128 * j + 32 * a:128 * j + 32 * (a + 1)],
                            recipT[32 * a:32 * (a + 1), j:j + 1],
                        )
                    nc.vector.tensor_copy(
                        compact_bf[:, 32 * j:32 * j + 32],
                        compact[:, 32 * j:32 * j + 32])
                    for a in range(4):
                        nc.tensor.matmul(
                            mt_ps[32 * a:32 * (a + 1), :],
                            compact_bf[32 * a:32 * (a + 1),
                                        32 * j:32 * j + 32],
                            wo_t[32 * a:32 * (a + 1), 256 * j:256 * (j + 1)],
                            start=(j == 0), stop=(j == 1),
                            tile_position=(32 * a, 32 * a),
                        )

                def qchunk(i):
                    base = i * 512
                    x_t = iopool.tile([128, 1024], BF16, tag="xchunk",
                                      name=f"x_{i}")
                    for k in range(2):
                        dma_eng = nc.sync if (i + k) % 2 == 0 else nc.gpsimd
                        dma_eng.dma_start(
                            x_t[:, 512 * k:512 * (k + 1)],
                            x[128 * k:128 * (k + 1), base:base + 512])
                    q_ps = psQ.tile([128, 512], F32, tag="q", name=f"q_{i}")
                    for k in range(2):
                        nc.tensor.matmul(
                            q_ps[:], wq_t[:, 128 * k:128 * (k + 1)],
                            x_t[:, 512 * k:512 * (k + 1)],
                            start=(k == 0), stop=(k == 1),
                        )
                    nc.scalar.activation(
                        expq[:, base:base + 512], q_ps[:], AF.Exp)

                def zqchunk(tc4):
                    zq_ps = psQ.tile([128, 512], F32, tag="q", name=f"zq_{tc4}")
                    for u in range(4):
                        t = 4 * tc4 + u
                        nc.tensor.matmul(
                            zq_ps[32 * u:32 * u + 32, :], ones4_t[:],
                            expq[:, 512 * t:512 * (t + 1)],
                            start=True, stop=True,
                            tile_position=(0, 32 * u),
                        )
                    zq_f = wkpool.tile([128, 512], F32, tag="zq_f",
                                       name=f"zqf_{tc4}")
                    nc.vector.reciprocal_approx_fast(zq_f[:], zq_ps[:])
                    nc.vector.tensor_copy(
                        zqr[:, 512 * tc4:512 * (tc4 + 1)], zq_f[:])

                # ---- branch 0: kv chunks with trailing ctx/zk batches ----
                for ch in range(nch):
                    pass1(0, ch)
                    if ch > 0:
                        pass2(0, ch - 1)
                pass2(0, nch - 1)
                compact_j(0)

                # ---- branch 1 interleaved with q projection + zq ----
                for ch in range(nch):
                    pass1(1, ch)
                    if ch > 0:
                        pass2(1, ch - 1)
                    qchunk(ch)
                    if ch % 4 == 3:
                        zqchunk(ch // 4)
                        for t in range(4 * (ch // 4), 4 * (ch // 4) + 4):
                            u, tc4 = t % 4, t // 4
                            zqb_ps = psQ.tile([128, 512], F32, tag="q",
                                              name=f"zqb_{t}")
                            nc.tensor.matmul(
                                zqb_ps[:], ones4T_t[32 * u:32 * u + 4, :],
                                zqr[32 * u:32 * u + 4,
                                    512 * tc4:512 * (tc4 + 1)],
                                start=True, stop=True,
                                tile_position=(32 * u, 0),
                            )
                            nc.vector.tensor_mul(
                                expq[:, 512 * t:512 * (t + 1)],
                                expq[:, 512 * t:512 * (t + 1)],
                                zqb_ps[:],
                            )
                pass2(1, nch - 1)
                compact_j(1)
                nc.vector.tensor_copy(mt_sb[:], mt_ps[:])

            # ======= Phase C: y tile = MT^T @ q-hat, 2 matmuls per tile ====
            with tc.tile_pool(name="psC", bufs=2, space="PSUM") as psC:
                for t in range(nt512):
                    y_sb = wkpool.tile([128, 2 * 512], BF16, tag="ysb",
                                       name=f"ysb_{t}")
                    for m in range(2):
                        y_ps = psC.tile([128, 512], F32, tag="y_ps",
                                        name=f"yps_{t}_{m}", bufs=4)
                        nc.tensor.matmul(
                            y_ps[:], mt_sb[:, 128 * m:128 * (m + 1)],
                            expq[:, 512 * t:512 * (t + 1)],
                            start=True, stop=True,
                        )
                        if m == 0:
                            nc.vector.tensor_copy(
                                y_sb[:, 512 * m:512 * (m + 1)], y_ps[:])
                        else:
                            nc.scalar.copy(
                                y_sb[:, 512 * m:512 * (m + 1)], y_ps[:])
                    for m in range(2):
                        dma_eng = nc.sync if (t + m) % 2 == 0 else nc.gpsimd
                        dma_eng.dma_start(
                            y[128 * m:128 * (m + 1), 512 * t:512 * (t + 1)],
                            y_sb[:, 512 * m:512 * (m + 1)])

    nc.compile()
    return nc


def _consts():
    ones4 = np.zeros((128, 32), dtype=ml_dtypes.bfloat16)
    for col in range(32):
        a = col % 4
        ones4[32 * a:32 * (a + 1), col] = 1
    ones4T = np.zeros((128, 128), dtype=ml_dtypes.bfloat16)
    for u in range(4):
        for a in range(4):
            ones4T[32 * u + a, 32 * a:32 * (a + 1)] = 1
    return ones4, ones4T


def _ktile(wT):
    """[C_in, C_out] -> [128, (C_in//128)*C_out] k-tiles along the free dim."""
    kin = wT.shape[0] // 128
    return np.concatenate([wT[128 * k:128 * (k + 1), :] for k in range(kin)], axis=1)


def make_in_maps(x, ref_1, ref_2, Wq, Wk1, Wk2, Wv1, Wv2, Wo, n_loc=N):
    bf = ml_dtypes.bfloat16
    ones4, ones4T = _consts()
    xf = np.asarray(x).reshape(B, C, -1)
    r1f = np.asarray(ref_1).reshape(B, C, -1)
    r2f = np.asarray(ref_2).reshape(B, C, -1)
    WqT, WoT = np.asarray(Wq).T, np.asarray(Wo).T
    WkT = [np.asarray(Wk1).T, np.asarray(Wk2).T]
    WvT = [np.asarray(Wv1).T, np.asarray(Wv2).T]
    gw = {}
    for g in range(2):
        sl = slice(128 * g, 128 * (g + 1))
        wq_g = np.ascontiguousarray(_ktile(WqT[:, sl])).astype(bf)
        wkv_g = [np.ascontiguousarray(
            _ktile(np.concatenate([WkT[j][:, sl], WvT[j][:, sl]], axis=1))
        ).astype(bf) for j in range(2)]
        # Wo rows for our concat channels: branch1 128g.., branch2 256+128g..
        wo_rows = np.concatenate(
            [WoT[sl, :], WoT[256 + 128 * g:256 + 128 * (g + 1), :]], axis=0)
        wo_g = np.ascontiguousarray(_ktile(wo_rows)).astype(bf)
        gw[g] = (wq_g, wkv_g[0], wkv_g[1], wo_g)
    in_maps = []
    for core in range(N_CORES):
        b, g = core // 2, core % 2
        wq_g, wkv1_g, wkv2_g, wo_g = gw[g]
        in_maps.append({
            "x": np.ascontiguousarray(xf[b, :, :n_loc]).astype(bf),
            "r1": np.ascontiguousarray(r1f[b, :, :n_loc]).astype(bf),
            "r2": np.ascontiguousarray(r2f[b, :, :n_loc]).astype(bf),
            "wq": wq_g, "wkv1": wkv1_g, "wkv2": wkv2_g, "wo": wo_g,
            "ones4": ones4, "ones4T": ones4T,
        })
    return in_maps


_NC_CACHE = {}


def kernel(x, ref_1, ref_2, Wq, Wk1, Wk2, Wv1, Wv2, Wo, _trace=False):
    n_loc = N
    if n_loc not in _NC_CACHE:
        _NC_CACHE[n_loc] = build_nc(n_loc)
    nc = _NC_CACHE[n_loc]
    in_maps = make_in_maps(x, ref_1, ref_2, Wq, Wk1, Wk2, Wv1, Wv2, Wo, n_loc)
    res = run_bass_kernel_spmd(nc, in_maps, core_ids=list(range(N_CORES)),
                               trace=_trace)
    out = np.empty((B, C, n_loc), dtype=np.float32)
    for b in range(B):
        out[b] = (res.results[2 * b]["y"].astype(np.float32)
                  + res.results[2 * b + 1]["y"].astype(np.float32))
    if _trace:
        kernel.last_results = res
    return out.reshape(B, C, H, W)



# revision 39
# speedup vs baseline: 1.2207x; 1.2207x over previous
"""Bidirectional cross-attention kernel for Trainium2, SPMD over 8 NeuronCores.

Reference (per batch b, heads K=8, head dim D=32, N=128*128 pixels):
    q   = softmax_d(Wq @ x)
    for branch j in {1,2}:
        key   = softmax_n(Wk_j @ ref_j)          # softmax over the pixel dim
        v     = Wv_j @ ref_j
        ctx_j = key @ v^T                        # [K,D,D]
        out_j = per-pixel  q @ ctx_j^T
    y = Wo @ concat(out_1, out_2)

Sharding: 8 cores = batch(4) x head-group(2).  Each core owns 4 of the 8
heads for its batch: projections, softmaxes, ctx and the out einsum are
fully head-local; the final Wo projection is computed as a partial sum
over the core's 256 (of 512) concat channels, and the host adds the two
partial outputs per batch.  No cross-core communication on device.

Numerics: bf16 matmul inputs (host-cast), fp32 PSUM accumulation, fp32
scalar/vector math.  Softmaxes skip max-subtraction (logits ~N(0,1), exp
is safe in fp32).

SBUF layout: tensors with >128 channels are stored as [128, k*cols] with
128-channel k-tiles side by side in the free dim.  Key/value tensors are
kept in transposed [pixel, channel] layout (needed for the ctx einsum,
whose contraction runs over pixels).
"""

import numpy as np
import ml_dtypes

import concourse.bass as bass
import concourse.bacc as bacc
import concourse.tile as tile
from concourse import mybir
from concourse.bass_utils import run_bass_kernel_spmd

BF16 = mybir.dt.bfloat16
F16 = mybir.dt.float16
F32 = mybir.dt.float32
AF = mybir.ActivationFunctionType

B, C, H, W = 4, 256, 128, 128
K, D = 8, 32
N = H * W
N_CORES = 8


def build_nc(n_loc=N):
    nc = bacc.Bacc("TRN2", target_bir_lowering=False, debug=False,
                   num_devices=N_CORES)

    nt = n_loc // 128        # 128-pixel tiles (128)
    nt512 = n_loc // 512     # 512-pixel tiles (32)

    # ---- I/O (weights pre-transposed, head-group-sliced, k-tiled on host) --
    x = nc.declare_dram_parameter("x", [C, n_loc], BF16, isOutput=False)
    r1 = nc.declare_dram_parameter("r1", [C, n_loc], BF16, isOutput=False)
    r2 = nc.declare_dram_parameter("r2", [C, n_loc], BF16, isOutput=False)
    # wq: [128, 2*128]  col chunk 128k = Wq.T[128k:128k+128, our 128 channels]
    wq = nc.declare_dram_parameter("wq", [128, 2 * 128], BF16, isOutput=False)
    # wkv_j: [128, 2*256] col chunk 256k = [WkT | WvT](our heads)[128k:, :]
    wkv1 = nc.declare_dram_parameter("wkv1", [128, 2 * 256], BF16, isOutput=False)
    wkv2 = nc.declare_dram_parameter("wkv2", [128, 2 * 256], BF16, isOutput=False)
    # wo: [128, 2*256]  col chunk 256k = Wo.T[our 256 concat channels][128k:, :]
    wo = nc.declare_dram_parameter("wo", [128, 2 * 256], BF16, isOutput=False)
    ones4 = nc.declare_dram_parameter("ones4", [128, 32], F16, isOutput=False)
    ones4T = nc.declare_dram_parameter("ones4T", [128, 128], F16, isOutput=False)

    y = nc.declare_dram_parameter("y", [C, n_loc], BF16, isOutput=True)

    refs = [r1, r2]

    with tile.TileContext(nc) as tc:
        with (
            tc.tile_pool(name="weights", bufs=1) as wpool,
            tc.tile_pool(name="persist", bufs=1) as ppool,
            tc.tile_pool(name="io", bufs=3) as iopool,
            tc.tile_pool(name="work", bufs=3) as wkpool,
        ):
            # ---- weights / constants (wkv1 first on sync so branch-0 can
            # start; everything else rides the gpsimd queue) ----
            wkv_t = []
            for j, wsrc in enumerate((wkv1, wkv2)):
                t = wpool.tile([128, 2 * 256], BF16, tag=f"wkv{j}", name=f"wkv_t{j}")
                (nc.sync if j == 0 else nc.gpsimd).dma_start(t[:], wsrc[:, :])
                wkv_t.append(t)
            wq_t = wpool.tile([128, 2 * 128], BF16, tag="wq")
            nc.gpsimd.dma_start(wq_t[:], wq[:, :])
            ones4_t = wpool.tile([128, 32], F16, tag="o4")
            nc.gpsimd.dma_start(ones4_t[:], ones4[:, :])
            ones4T_t = wpool.tile([128, 128], F16, tag="o4T")
            nc.gpsimd.dma_start(ones4T_t[:], ones4T[:, :])
            wo_t = wpool.tile([128, 2 * 256], BF16, tag="wo")
            nc.gpsimd.dma_start(wo_t[:], wo[:, :])

            compact = ppool.tile([128, 64], F32, tag="compact")
            compact_bf = ppool.tile([128, 64], BF16, tag="compact_bf")
            mt_sb = ppool.tile([128, 256], F16, tag="mt_sb")
            recipT = ppool.tile([128, 2], F32, tag="recipT")
            expq = ppool.tile([128, n_loc], F16, tag="expq")
            nzc = (nt512 + 3) // 4
            zqr = ppool.tile([128, 512 * nzc], F16, tag="zqr")

            CH = 4               # kv: 128-pixel tiles per chunk
            nch = nt // CH       # 32 chunks per branch

            with (
                tc.tile_pool(name="kvstage", bufs=1) as kvpool,
                tc.tile_pool(name="psA", bufs=2, space="PSUM") as psA,
                tc.tile_pool(name="psAcc", bufs=1, space="PSUM") as psAcc,
                tc.tile_pool(name="psQ", bufs=2, space="PSUM") as psQ,
            ):
                ekt_all = kvpool.tile([128, nt * 128], BF16, tag="ekt_all")
                vt_all = kvpool.tile([128, nt * 128], BF16, tag="vt_all")
                ones1_t = wpool.tile([128, 1], BF16, tag="o1")
                nc.vector.memset(ones1_t[:], 1.0)
                # cols 128j..128j+128 = branch j ctx[c,d]
                ctx_ps = psAcc.tile([128, 256], F32, tag="ctx")
                # zkT col j = sum_n exp(k_j[c,n]) on partitions c
                zkT_ps = psAcc.tile([128, 2], F32, tag="zkT")

                def pass1(j, ch):
                    base = ch * CH * 128
                    r_t = iopool.tile([128, CH * 256], BF16, tag="rchunk",
                                      name=f"r_{j}_{ch}")
                    for k in range(2):
                        dma_eng = nc.sync if (ch + k) % 2 == 0 else nc.gpsimd
                        dma_eng.dma_start(
                            r_t[:, CH * 128 * k:CH * 128 * (k + 1)],
                            refs[j][128 * k:128 * (k + 1), base:base + CH * 128])
                    kv_ps = psA.tile([128, CH * 256], F32, tag="kv",
                                     name=f"kv_{j}_{ch}")
                    for t in range(CH):
                        for k in range(2):
                            nc.tensor.matmul(
                                kv_ps[:, 256 * t:256 * (t + 1)],
                                r_t[:, CH * 128 * k + 128 * t:
                                       CH * 128 * k + 128 * (t + 1)],
                                wkv_t[j][:, 256 * k:256 * (k + 1)],
                                start=(k == 0), stop=(k == 1),
                            )
                    ek_sl = ekt_all[:, ch * CH * 128:(ch + 1) * CH * 128]
                    nc.scalar.activation(
                        ek_sl.rearrange("p (t c) -> p t c", t=CH),
                        kv_ps[:].rearrange("p (t c) -> p t c", t=CH)[:, :, 0:128],
                        AF.Exp,
                    )
                    vt_sl = vt_all[:, ch * CH * 128:(ch + 1) * CH * 128]
                    nc.vector.tensor_copy(
                        vt_sl.rearrange("p (t c) -> p t c", t=CH),
                        kv_ps[:].rearrange("p (t c) -> p t c", t=CH)[:, :, 128:256],
                    )

                def pass2(j, ch):
                    # ctx[c,d] accumulation; the N=1 ones-matmul reuses the
                    # same ekt stationary to accumulate zkT[c] nearly free.
                    for t in range(ch * CH, (ch + 1) * CH):
                        nc.tensor.matmul(
                            ctx_ps[:, 128 * j:128 * (j + 1)],
                            ekt_all[:, 128 * t:128 * (t + 1)],
                            vt_all[:, 128 * t:128 * (t + 1)],
                            start=(t == 0), stop=(t == nt - 1),
                        )
                        nc.tensor.matmul(
                            zkT_ps[:, j:j + 1],
                            ekt_all[:, 128 * t:128 * (t + 1)],
                            ones1_t[:],
                            start=(t == 0), stop=(t == nt - 1),
                        )

                def compact_j(j):
                    # fold 1/zk[c] into the diag 32x32 blocks of ctx[c,d]
                    # (c on partitions), cast to bf16
                    nc.vector.reciprocal_approx_fast(
                        recipT[:, j:j + 1], zkT_ps[:, j:j + 1])
                    for a in range(4):
                        nc.vector.tensor_scalar_mul(
                            compact[32 * a:32 * (a + 1), 32 * j:32 * j + 32],
                            ctx_ps[32 * a:32 * (a + 1),
                                   128 * j + 32 * a:128 * j + 32 * (a + 1)],
                            recipT[32 * a:32 * (a + 1), j:j + 1],
                        )
                    nc.vector.tensor_copy(
                        compact_bf[:, 32 * j:32 * j + 32],
                        compact[:, 32 * j:32 * j + 32])

                def qchunk(i):
                    base = i * 512
                    x_t = iopool.tile([128, 1024], BF16, tag="xchunk",
                                      name=f"x_{i}")
                    for k in range(2):
                        dma_eng = nc.sync if (i + k) % 2 == 0 else nc.gpsimd
                        dma_eng.dma_start(
                            x_t[:, 512 * k:512 * (k + 1)],
                            x[128 * k:128 * (k + 1), base:base + 512])
                    q_ps = psQ.tile([128, 512], F32, tag="q", name=f"q_{i}")
                    for k in range(2):
                        nc.tensor.matmul(
                            q_ps[:], wq_t[:, 128 * k:128 * (k + 1)],
                            x_t[:, 512 * k:512 * (k + 1)],
                            start=(k == 0), stop=(k == 1),
                        )
                    nc.scalar.activation(
                        expq[:, base:base + 512], q_ps[:], AF.Exp)

                def zqchunk(tc4):
                    zq_ps = psQ.tile([128, 512], F32, tag="q", name=f"zq_{tc4}")
                    for u in range(4):
                        t = 4 * tc4 + u
                        nc.tensor.matmul(
                            zq_ps[32 * u:32 * u + 32, :], ones4_t[:],
                            expq[:, 512 * t:512 * (t + 1)],
                            start=True, stop=True,
                            tile_position=(0, 32 * u),
                        )
                    zq_f = wkpool.tile([128, 512], F32, tag="zq_f",
                                       name=f"zqf_{tc4}")
                    nc.vector.reciprocal_approx_fast(zq_f[:], zq_ps[:])
                    nc.vector.tensor_copy(
                        zqr[:, 512 * tc4:512 * (tc4 + 1)], zq_f[:])

                # ---- branch 0: kv chunks with trailing ctx/zk batches ----
                for ch in range(nch):
                    pass1(0, ch)
                    if ch > 0:
                        pass2(0, ch - 1)
                pass2(0, nch - 1)
                compact_j(0)

                # ---- branch 1 interleaved with q projection + zq ----
                for ch in range(nch):
                    pass1(1, ch)
                    if ch > 0:
                        pass2(1, ch - 1)
                    qchunk(ch)
                    if ch % 4 == 3:
                        zqchunk(ch // 4)
                        for t in range(4 * (ch // 4), 4 * (ch // 4) + 4):
                            u, tc4 = t % 4, t // 4
                            zqb_ps = psQ.tile([128, 512], F32, tag="q",
                                              name=f"zqb_{t}")
                            nc.tensor.matmul(
                                zqb_ps[:], ones4T_t[32 * u:32 * u + 4, :],
                                zqr[32 * u:32 * u + 4,
                                    512 * tc4:512 * (tc4 + 1)],
                                start=True, stop=True,
                                tile_position=(32 * u, 0),
                            )
                            nc.vector.tensor_mul(
                                expq[:, 512 * t:512 * (t + 1)],
                                expq[:, 512 * t:512 * (t + 1)],
                                zqb_ps[:],
                            )
                pass2(1, nch - 1)
                compact_j(1)

            # MT[d, o] = sum_j sum_c ctx_j[c,d]/zk_j[c] * WoT_j[c, o];
            # head-block-diagonal, so 4 PE-packed matmuls per branch.  Kept
            # as f32r so the y matmul loses no precision on the weights.
            with tc.tile_pool(name="psM", bufs=1, space="PSUM") as psM:
                mt_ps = psM.tile([128, 256], F32, tag="mt")
                for j in range(2):
                    for a in range(4):
                        nc.tensor.matmul(
                            mt_ps[32 * a:32 * (a + 1), :],
                            compact_bf[32 * a:32 * (a + 1),
                                       32 * j:32 * j + 32],
                            wo_t[32 * a:32 * (a + 1), 256 * j:256 * (j + 1)],
                            start=(j == 0), stop=(j == 1),
                            tile_position=(32 * a, 32 * a),
                        )
                nc.vector.tensor_copy(mt_sb[:], mt_ps[:])

            # ======= Phase C: y tile = MT^T @ q-hat, 2 matmuls per tile ====
            # Tiles processed in pairs so each half-row y DMA moves 2 KB
            # per partition (half the dispatch count).
            with (
                tc.tile_pool(name="psC", bufs=6, space="PSUM") as psC,
                tc.tile_pool(name="ysb", bufs=4) as ypool,
            ):
                for g in range(nt512 // 2):
                    y_sb = ypool.tile([128, 2048], BF16, tag="ysb",
                                      name=f"ysb_{g}")
                    for u in range(2):
                        t = 2 * g + u
                        for m in range(2):
                            y_ps = psC.tile([128, 512], F32, tag="y_ps",
                                            name=f"yps_{t}_{m}", bufs=6)
                            nc.tensor.matmul(
                                y_ps[:], mt_sb[:, 128 * m:128 * (m + 1)],
                                expq[:, 512 * t:512 * (t + 1)],
                                start=True, stop=True,
                            )
                            csl = y_sb[:, 1024 * m + 512 * u:
                                          1024 * m + 512 * (u + 1)]
                            if m == 0:
                                nc.vector.tensor_copy(csl, y_ps[:])
                            else:
                                nc.scalar.copy(csl, y_ps[:])
                    for m in range(2):
                        dma_eng = nc.sync if (g + m) % 2 == 0 else nc.gpsimd
                        dma_eng.dma_start(
                            y[128 * m:128 * (m + 1), 1024 * g:1024 * (g + 1)],
                            y_sb[:, 1024 * m:1024 * (m + 1)])

    nc.compile()
    return nc


def _consts():
    ones4 = np.zeros((128, 32), dtype=np.float16)
    for col in range(32):
        a = col % 4
        ones4[32 * a:32 * (a + 1), col] = 1
    ones4T = np.zeros((128, 128), dtype=np.float16)
    for u in range(4):
        for a in range(4):
            ones4T[32 * u + a, 32 * a:32 * (a + 1)] = 1
    return ones4, ones4T


def _ktile(wT):
    """[C_in, C_out] -> [128, (C_in//128)*C_out] k-tiles along the free dim."""
    kin = wT.shape[0] // 128
    return np.concatenate([wT[128 * k:128 * (k + 1), :] for k in range(kin)], axis=1)


def make_in_maps(x, ref_1, ref_2, Wq, Wk1, Wk2, Wv1, Wv2, Wo, n_loc=N):
    bf = ml_dtypes.bfloat16
    ones4, ones4T = _consts()
    xf = np.asarray(x).reshape(B, C, -1)
    r1f = np.asarray(ref_1).reshape(B, C, -1)
    r2f = np.asarray(ref_2).reshape(B, C, -1)
    WqT, WoT = np.asarray(Wq).T, np.asarray(Wo).T
    WkT = [np.asarray(Wk1).T, np.asarray(Wk2).T]
    WvT = [np.asarray(Wv1).T, np.asarray(Wv2).T]
    gw = {}
    for g in range(2):
        sl = slice(128 * g, 128 * (g + 1))
        wq_g = np.ascontiguousarray(_ktile(WqT[:, sl])).astype(bf)
        wkv_g = [np.ascontiguousarray(
            _ktile(np.concatenate([WkT[j][:, sl], WvT[j][:, sl]], axis=1))
        ).astype(bf) for j in range(2)]
        # Wo rows for our concat channels: branch1 128g.., branch2 256+128g..
        wo_rows = np.concatenate(
            [WoT[sl, :], WoT[256 + 128 * g:256 + 128 * (g + 1), :]], axis=0)
        wo_g = np.ascontiguousarray(_ktile(wo_rows)).astype(bf)
        gw[g] = (wq_g, wkv_g[0], wkv_g[1], wo_g)
    in_maps = []
    for core in range(N_CORES):
        b, g = core // 2, core % 2
        wq_g, wkv1_g, wkv2_g, wo_g = gw[g]
        in_maps.append({
            "x": np.ascontiguousarray(xf[b, :, :n_loc]).astype(bf),
            "r1": np.ascontiguousarray(r1f[b, :, :n_loc]).astype(bf),
            "r2": np.ascontiguousarray(r2f[b, :, :n_loc]).astype(bf),
            "wq": wq_g, "wkv1": wkv1_g, "wkv2": wkv2_g, "wo": wo_g,
            "ones4": ones4, "ones4T": ones4T,
        })
    return in_maps


_NC_CACHE = {}


def kernel(x, ref_1, ref_2, Wq, Wk1, Wk2, Wv1, Wv2, Wo, _trace=False):
    n_loc = N
    if n_loc not in _NC_CACHE:
        _NC_CACHE[n_loc] = build_nc(n_loc)
    nc = _NC_CACHE[n_loc]
    in_maps = make_in_maps(x, ref_1, ref_2, Wq, Wk1, Wk2, Wv1, Wv2, Wo, n_loc)
    res = run_bass_kernel_spmd(nc, in_maps, core_ids=list(range(N_CORES)),
                               trace=_trace)
    out = np.empty((B, C, n_loc), dtype=np.float32)
    for b in range(B):
        out[b] = (res.results[2 * b]["y"].astype(np.float32)
                  + res.results[2 * b + 1]["y"].astype(np.float32))
    if _trace:
        kernel.last_results = res
    return out.reshape(B, C, H, W)



# revision 71
# speedup vs baseline: 1.2553x; 1.0284x over previous
"""Bidirectional cross-attention kernel for Trainium2, SPMD over 8 NeuronCores.

Reference (per batch b, heads K=8, head dim D=32, N=128*128 pixels):
    q   = softmax_d(Wq @ x)
    for branch j in {1,2}:
        key   = softmax_n(Wk_j @ ref_j)          # softmax over the pixel dim
        v     = Wv_j @ ref_j
        ctx_j = key @ v^T                        # [K,D,D]
        out_j = per-pixel  q @ ctx_j^T
    y = Wo @ concat(out_1, out_2)

Sharding: 8 cores = batch(4) x head-group(2).  Each core owns 4 of the 8
heads for its batch: projections, softmaxes, ctx and the out einsum are
fully head-local; the final Wo projection is computed as a partial sum
over the core's 256 (of 512) concat channels, and the host adds the two
partial outputs per batch.  No cross-core communication on device.

Numerics: bf16 matmul inputs (host-cast), fp32 PSUM accumulation, fp32
scalar/vector math.  Softmaxes skip max-subtraction (logits ~N(0,1), exp
is safe in fp32).

SBUF layout: tensors with >128 channels are stored as [128, k*cols] with
128-channel k-tiles side by side in the free dim.  Key/value tensors are
kept in transposed [pixel, channel] layout (needed for the ctx einsum,
whose contraction runs over pixels).
"""

import numpy as np
import ml_dtypes

import concourse.bass as bass
import concourse.bacc as bacc
import concourse.tile as tile
from concourse import mybir
from concourse.bass_utils import run_bass_kernel_spmd

BF16 = mybir.dt.bfloat16
F16 = mybir.dt.float16
F32 = mybir.dt.float32
FP8 = mybir.dt.float8e4
DOUBLE_ROW = mybir.MatmulPerfMode.DoubleRow
AF = mybir.ActivationFunctionType

B, C, H, W = 4, 256, 128, 128
K, D = 8, 32
N = H * W
N_CORES = 8


def build_nc(n_loc=N):
    nc = bacc.Bacc("TRN2", target_bir_lowering=False, debug=False,
                   num_devices=N_CORES)

    nt = n_loc // 128        # 128-pixel tiles (128)
    nt512 = n_loc // 512     # 512-pixel tiles (32)

    # ---- I/O (weights pre-transposed, head-group-sliced, k-tiled on host) --
    x = nc.declare_dram_parameter("x", [C, n_loc], BF16, isOutput=False)
    r1 = nc.declare_dram_parameter("r1", [C, n_loc], BF16, isOutput=False)
    r2 = nc.declare_dram_parameter("r2", [C, n_loc], BF16, isOutput=False)
    # wq: [128, 2*128]  col chunk 128k = Wq.T[128k:128k+128, our 128 channels]
    wq = nc.declare_dram_parameter("wq", [128, 2 * 128], BF16, isOutput=False)
    # wkv_j: [128, 2*256] col chunk 256k = [WkT | WvT](our heads)[128k:, :]
    wkv1 = nc.declare_dram_parameter("wkv1", [128, 2 * 256], BF16, isOutput=False)
    wkv2 = nc.declare_dram_parameter("wkv2", [128, 2 * 256], BF16, isOutput=False)
    # wo: [128, 2*256]  col chunk 256k = Wo.T[our 256 concat channels][128k:, :]
    wo = nc.declare_dram_parameter("wo", [128, 2 * 256], BF16, isOutput=False)
    ones4 = nc.declare_dram_parameter("ones4", [128, 32], F16, isOutput=False)
    ones4T = nc.declare_dram_parameter("ones4T", [128, 128], F16, isOutput=False)

    y = nc.declare_dram_parameter("y", [C, n_loc], BF16, isOutput=True)

    refs = [r1, r2]

    with tile.TileContext(nc) as tc:
        with (
            tc.tile_pool(name="weights", bufs=1) as wpool,
            tc.tile_pool(name="persist", bufs=1) as ppool,
            tc.tile_pool(name="io", bufs=3) as iopool,
            tc.tile_pool(name="work", bufs=3) as wkpool,
        ):
            # ---- weights / constants (wkv1 first on sync so branch-0 can
            # start; everything else rides the gpsimd queue) ----
            wkv_t = []
            for j, wsrc in enumerate((wkv1, wkv2)):
                t = wpool.tile([128, 2 * 256], BF16, tag=f"wkv{j}", name=f"wkv_t{j}")
                (nc.sync if j == 0 else nc.gpsimd).dma_start(t[:], wsrc[:, :])
                wkv_t.append(t)
            wq_t = wpool.tile([128, 2 * 128], BF16, tag="wq")
            nc.gpsimd.dma_start(wq_t[:], wq[:, :])
            ones4_t = wpool.tile([128, 32], F16, tag="o4")
            nc.gpsimd.dma_start(ones4_t[:], ones4[:, :])
            ones4T_t = wpool.tile([128, 128], F16, tag="o4T")
            nc.gpsimd.dma_start(ones4T_t[:], ones4T[:, :])
            wo_t = wpool.tile([128, 2 * 256], BF16, tag="wo")
            nc.gpsimd.dma_start(wo_t[:], wo[:, :])

            compact = ppool.tile([128, 64], F32, tag="compact")
            compact_bf = ppool.tile([128, 64], BF16, tag="compact_bf")
            mt_sb = ppool.tile([128, 256], F16, tag="mt_sb")
            recipT = ppool.tile([128, 2], F32, tag="recipT")
            expq = ppool.tile([128, n_loc], F16, tag="expq")
            nzc = (nt512 + 3) // 4
            zqr = ppool.tile([128, 512 * nzc], F16, tag="zqr")

            CH = 4               # kv: 128-pixel tiles per chunk
            nch = nt // CH       # 32 chunks per branch

            with (
                tc.tile_pool(name="kvstage", bufs=1) as kvpool,
                tc.tile_pool(name="psA", bufs=2, space="PSUM") as psA,
                tc.tile_pool(name="psAcc", bufs=1, space="PSUM") as psAcc,
                tc.tile_pool(name="psQ", bufs=2, space="PSUM") as psQ,
            ):
                # ek/vt live in fp8: per-pixel quantization noise averages
                # out over the 16384-pixel ctx contraction.  exp is biased
                # by -3 so ek stays inside fp8 e4m3 range; the bias cancels
                # exactly in ctx/zk.
                ekt_all = kvpool.tile([128, nt * 128], FP8, tag="ekt_all")
                vt_all = kvpool.tile([128, nt * 128], FP8, tag="vt_all")
                ones1_t = wpool.tile([128, 2], FP8, tag="o1")
                nc.vector.memset(ones1_t[:], 1.0)
                bias3 = wpool.tile([128, 1], F32, tag="bias3")
                nc.vector.memset(bias3[:], -3.0)
                # cols 128j..128j+128 = branch j ctx[c,d]
                ctx_ps = psAcc.tile([128, 256], F32, tag="ctx")
                # zkT col j = sum_n exp(k_j[c,n]) on partitions c
                zkT_ps = psAcc.tile([128, 2], F32, tag="zkT")

                def pass1(j, ch):
                    base = ch * CH * 128
                    r_t = iopool.tile([128, CH * 256], BF16, tag="rchunk",
                                      name=f"r_{j}_{ch}")
                    for k in range(2):
                        dma_eng = nc.sync if (ch + k) % 2 == 0 else nc.gpsimd
                        dma_eng.dma_start(
                            r_t[:, CH * 128 * k:CH * 128 * (k + 1)],
                            refs[j][128 * k:128 * (k + 1), base:base + CH * 128])
                    kv_ps = psA.tile([128, CH * 256], F32, tag="kv",
                                     name=f"kv_{j}_{ch}")
                    for t in range(CH):
                        for k in range(2):
                            nc.tensor.matmul(
                                kv_ps[:, 256 * t:256 * (t + 1)],
                                r_t[:, CH * 128 * k + 128 * t:
                                       CH * 128 * k + 128 * (t + 1)],
                                wkv_t[j][:, 256 * k:256 * (k + 1)],
                                start=(k == 0), stop=(k == 1),
                            )
                    ek_sl = ekt_all[:, ch * CH * 128:(ch + 1) * CH * 128]
                    nc.scalar.activation(
                        ek_sl.rearrange("p (t c) -> p t c", t=CH),
                        kv_ps[:].rearrange("p (t c) -> p t c", t=CH)[:, :, 0:128],
                        AF.Exp, bias=bias3[:],
                    )
                    vt_sl = vt_all[:, ch * CH * 128:(ch + 1) * CH * 128]
                    nc.vector.tensor_copy(
                        vt_sl.rearrange("p (t c) -> p t c", t=CH),
                        kv_ps[:].rearrange("p (t c) -> p t c", t=CH)[:, :, 128:256],
                    )

                def pass2(j, ch):
                    # fp8 DoubleRow over tile pairs; the N=1 ones-matmul
                    # reuses the same ekt stationary to accumulate zkT[c].
                    for tp in range(ch * CH // 2, (ch + 1) * CH // 2):
                        ek_pair = ekt_all[:, 256 * tp:256 * (tp + 1)].rearrange(
                            "p (k c) -> p k c", k=2)
                        nc.tensor.matmul(
                            ctx_ps[:, 128 * j:128 * (j + 1)],
                            ek_pair,
                            vt_all[:, 256 * tp:256 * (tp + 1)].rearrange(
                                "p (k c) -> p k c", k=2),
                            start=(tp == 0), stop=(tp == nt // 2 - 1),
                            perf_mode=DOUBLE_ROW,
                        )
                        nc.tensor.matmul(
                            zkT_ps[:, j:j + 1],
                            ek_pair,
                            ones1_t[:].rearrange("p (k c) -> p k c", k=2),
                            start=(tp == 0), stop=(tp == nt // 2 - 1),
                            perf_mode=DOUBLE_ROW,
                        )

                def compact_j(j):
                    # fold 1/zk[c] into the diag 32x32 blocks of ctx[c,d]
                    # (c on partitions), cast to bf16
                    nc.vector.reciprocal_approx_fast(
                        recipT[:, j:j + 1], zkT_ps[:, j:j + 1])
                    for a in range(4):
                        nc.vector.tensor_scalar_mul(
                            compact[32 * a:32 * (a + 1), 32 * j:32 * j + 32],
                            ctx_ps[32 * a:32 * (a + 1),
                                   128 * j + 32 * a:128 * j + 32 * (a + 1)],
                            recipT[32 * a:32 * (a + 1), j:j + 1],
                        )
                    nc.vector.tensor_copy(
                        compact_bf[:, 32 * j:32 * j + 32],
                        compact[:, 32 * j:32 * j + 32])

                def qchunk(i):
                    base = i * 512
                    x_t = iopool.tile([128, 1024], BF16, tag="xchunk",
                                      name=f"x_{i}")
                    for k in range(2):
                        dma_eng = nc.sync if (i + k) % 2 == 0 else nc.gpsimd
                        dma_eng.dma_start(
                            x_t[:, 512 * k:512 * (k + 1)],
                            x[128 * k:128 * (k + 1), base:base + 512])
                    q_ps = psQ.tile([128, 512], F32, tag="q", name=f"q_{i}")
                    for k in range(2):
                        nc.tensor.matmul(
                            q_ps[:], wq_t[:, 128 * k:128 * (k + 1)],
                            x_t[:, 512 * k:512 * (k + 1)],
                            start=(k == 0), stop=(k == 1),
                        )
                    nc.scalar.activation(
                        expq[:, base:base + 512], q_ps[:], AF.Exp)

                def zqchunk(tc4):
                    zq_ps = psQ.tile([128, 512], F32, tag="q", name=f"zq_{tc4}")
                    for u in range(4):
                        t = 4 * tc4 + u
                        nc.tensor.matmul(
                            zq_ps[32 * u:32 * u + 32, :], ones4_t[:],
                            expq[:, 512 * t:512 * (t + 1)],
                            start=True, stop=True,
                            tile_position=(0, 32 * u),
                        )
                    zq_f = wkpool.tile([128, 512], F32, tag="zq_f",
                                       name=f"zqf_{tc4}")
                    nc.vector.reciprocal_approx_fast(zq_f[:], zq_ps[:])
                    nc.vector.tensor_copy(
                        zqr[:, 512 * tc4:512 * (tc4 + 1)], zq_f[:])

                # ---- branch 0: kv chunks with trailing ctx/zk batches ----
                for ch in range(nch):
                    pass1(0, ch)
                    if ch > 0:
                        pass2(0, ch - 1)
                pass2(0, nch - 1)
                compact_j(0)

                # ---- branch 1 interleaved with q projection + zq ----
                for ch in range(nch):
                    pass1(1, ch)
                    if ch > 0:
                        pass2(1, ch - 1)
                    qchunk(ch)
                    if ch % 4 == 3:
                        zqchunk(ch // 4)
                        for t in range(4 * (ch // 4), 4 * (ch // 4) + 4):
                            u, tc4 = t % 4, t // 4
                            zqb_ps = psQ.tile([128, 512], F32, tag="q",
                                              name=f"zqb_{t}")
                            # K=32 selector over the 8x-replicated recip rows
                            # streams faster than the K=4 broadcast
                            nc.tensor.matmul(
                                zqb_ps[:], ones4T_t[32 * u:32 * (u + 1), :],
                                zqr[32 * u:32 * (u + 1),
                                    512 * tc4:512 * (tc4 + 1)],
                                start=True, stop=True,
                                tile_position=(32 * u, 0),
                            )
                            nc.vector.tensor_mul(
                                expq[:, 512 * t:512 * (t + 1)],
                                expq[:, 512 * t:512 * (t + 1)],
                                zqb_ps[:],
                            )
                pass2(1, nch - 1)
                compact_j(1)

            # MT[d, o] = sum_j sum_c ctx_j[c,d]/zk_j[c] * WoT_j[c, o];
            # head-block-diagonal, so 4 PE-packed matmuls per branch.  Kept
            # as f32r so the y matmul loses no precision on the weights.
            with tc.tile_pool(name="psM", bufs=1, space="PSUM") as psM:
                mt_ps = psM.tile([128, 256], F32, tag="mt")
                for j in range(2):
                    for a in range(4):
                        nc.tensor.matmul(
                            mt_ps[32 * a:32 * (a + 1), :],
                            compact_bf[32 * a:32 * (a + 1),
                                       32 * j:32 * j + 32],
                            wo_t[32 * a:32 * (a + 1), 256 * j:256 * (j + 1)],
                            start=(j == 0), stop=(j == 1),
                            tile_position=(32 * a, 32 * a),
                        )
                nc.vector.tensor_copy(mt_sb[:], mt_ps[:])

            # ======= Phase C: y tile = MT^T @ q-hat, 2 matmuls per tile ====
            # Tiles processed in pairs so each half-row y DMA moves 2 KB
            # per partition (half the dispatch count).
            with (
                tc.tile_pool(name="psC", bufs=6, space="PSUM") as psC,
                tc.tile_pool(name="ysb", bufs=4) as ypool,
            ):
                for g in range(nt512 // 2):
                    y_sb = ypool.tile([128, 2048], BF16, tag="ysb",
                                      name=f"ysb_{g}")
                    for u in range(2):
                        t = 2 * g + u
                        for m in range(2):
                            y_ps = psC.tile([128, 512], F32, tag="y_ps",
                                            name=f"yps_{t}_{m}", bufs=6)
                            nc.tensor.matmul(
                                y_ps[:], mt_sb[:, 128 * m:128 * (m + 1)],
                                expq[:, 512 * t:512 * (t + 1)],
                                start=True, stop=True,
                            )
                            csl = y_sb[:, 1024 * m + 512 * u:
                                          1024 * m + 512 * (u + 1)]
                            if m == 0:
                                nc.vector.tensor_copy(csl, y_ps[:])
                            else:
                                nc.scalar.copy(csl, y_ps[:])
                    for m in range(2):
                        dma_eng = nc.sync if (g + m) % 2 == 0 else nc.gpsimd
                        dma_eng.dma_start(
                            y[128 * m:128 * (m + 1), 1024 * g:1024 * (g + 1)],
                            y_sb[:, 1024 * m:1024 * (m + 1)])

    nc.compile()
    return nc


def _consts():
    ones4 = np.zeros((128, 32), dtype=np.float16)
    for col in range(32):
        a = col % 4
        ones4[32 * a:32 * (a + 1), col] = 1
    # selector: row 32u + a + 4*(d%8) feeds output channel 32a + d, picking
    # one of the 8 identical per-head sum replicas per output row
    ones4T = np.zeros((128, 128), dtype=np.float16)
    for u in range(4):
        for a in range(4):
            for d in range(32):
                ones4T[32 * u + a + 4 * (d % 8), 32 * a + d] = 1
    return ones4, ones4T


def _ktile(wT):
    """[C_in, C_out] -> [128, (C_in//128)*C_out] k-tiles along the free dim."""
    kin = wT.shape[0] // 128
    return np.concatenate([wT[128 * k:128 * (k + 1), :] for k in range(kin)], axis=1)


def make_in_maps(x, ref_1, ref_2, Wq, Wk1, Wk2, Wv1, Wv2, Wo, n_loc=N):
    bf = ml_dtypes.bfloat16
    ones4, ones4T = _consts()
    xf = np.asarray(x).reshape(B, C, -1)
    r1f = np.asarray(ref_1).reshape(B, C, -1)
    r2f = np.asarray(ref_2).reshape(B, C, -1)
    WqT, WoT = np.asarray(Wq).T, np.asarray(Wo).T
    WkT = [np.asarray(Wk1).T, np.asarray(Wk2).T]
    WvT = [np.asarray(Wv1).T, np.asarray(Wv2).T]
    gw = {}
    for g in range(2):
        sl = slice(128 * g, 128 * (g + 1))
        wq_g = np.ascontiguousarray(_ktile(WqT[:, sl])).astype(bf)
        wkv_g = [np.ascontiguousarray(
            _ktile(np.concatenate([WkT[j][:, sl], WvT[j][:, sl]], axis=1))
        ).astype(bf) for j in range(2)]
        # Wo rows for our concat channels: branch1 128g.., branch2 256+128g..
        wo_rows = np.concatenate(
            [WoT[sl, :], WoT[256 + 128 * g:256 + 128 * (g + 1), :]], axis=0)
        wo_g = np.ascontiguousarray(_ktile(wo_rows)).astype(bf)
        gw[g] = (wq_g, wkv_g[0], wkv_g[1], wo_g)
    in_maps = []
    for core in range(N_CORES):
        b, g = core // 2, core % 2
        wq_g, wkv1_g, wkv2_g, wo_g = gw[g]
        in_maps.append({
            "x": np.ascontiguousarray(xf[b, :, :n_loc]).astype(bf),
            "r1": np.ascontiguousarray(r1f[b, :, :n_loc]).astype(bf),
            "r2": np.ascontiguousarray(r2f[b, :, :n_loc]).astype(bf),
            "wq": wq_g, "wkv1": wkv1_g, "wkv2": wkv2_g, "wo": wo_g,
            "ones4": ones4, "ones4T": ones4T,
        })
    return in_maps


_NC_CACHE = {}


def kernel(x, ref_1, ref_2, Wq, Wk1, Wk2, Wv1, Wv2, Wo, _trace=False):
    n_loc = N
    if n_loc not in _NC_CACHE:
        _NC_CACHE[n_loc] = build_nc(n_loc)
    nc = _NC_CACHE[n_loc]
    in_maps = make_in_maps(x, ref_1, ref_2, Wq, Wk1, Wk2, Wv1, Wv2, Wo, n_loc)
    res = run_bass_kernel_spmd(nc, in_maps, core_ids=list(range(N_CORES)),
                               trace=_trace)
    out = np.empty((B, C, n_loc), dtype=np.float32)
    for b in range(B):
        out[b] = (res.results[2 * b]["y"].astype(np.float32)
                  + res.results[2 * b + 1]["y"].astype(np.float32))
    if _trace:
        kernel.last_results = res
    return out.reshape(B, C, H, W)



# revision 79
# speedup vs baseline: 1.2644x; 1.0072x over previous
"""Bidirectional cross-attention kernel for Trainium2, SPMD over 8 NeuronCores.

Reference (per batch b, heads K=8, head dim D=32, N=128*128 pixels):
    q   = softmax_d(Wq @ x)
    for branch j in {1,2}:
        key   = softmax_n(Wk_j @ ref_j)          # softmax over the pixel dim
        v     = Wv_j @ ref_j
        ctx_j = key @ v^T                        # [K,D,D]
        out_j = per-pixel  q @ ctx_j^T
    y = Wo @ concat(out_1, out_2)

Sharding: 8 cores = batch(4) x head-group(2).  Each core owns 4 of the 8
heads for its batch: projections, softmaxes, ctx and the out einsum are
fully head-local; the final Wo projection is computed as a partial sum
over the core's 256 (of 512) concat channels, and the host adds the two
partial outputs per batch.  No cross-core communication on device.

Numerics: bf16 matmul inputs (host-cast), fp32 PSUM accumulation, fp32
scalar/vector math.  Softmaxes skip max-subtraction (logits ~N(0,1), exp
is safe in fp32).

SBUF layout: tensors with >128 channels are stored as [128, k*cols] with
128-channel k-tiles side by side in the free dim.  Key/value tensors are
kept in transposed [pixel, channel] layout (needed for the ctx einsum,
whose contraction runs over pixels).
"""

import numpy as np
import ml_dtypes

import concourse.bass as bass
import concourse.bacc as bacc
import concourse.tile as tile
from concourse import mybir
from concourse.bass_utils import run_bass_kernel_spmd

BF16 = mybir.dt.bfloat16
F16 = mybir.dt.float16
F32 = mybir.dt.float32
FP8 = mybir.dt.float8e4
DOUBLE_ROW = mybir.MatmulPerfMode.DoubleRow
AF = mybir.ActivationFunctionType

B, C, H, W = 4, 256, 128, 128
K, D = 8, 32
N = H * W
N_CORES = 8


def build_nc(n_loc=N):
    nc = bacc.Bacc("TRN2", target_bir_lowering=False, debug=False,
                   num_devices=N_CORES)

    nt = n_loc // 128        # 128-pixel tiles (128)
    nt512 = n_loc // 512     # 512-pixel tiles (32)

    # ---- I/O (weights pre-transposed, head-group-sliced, k-tiled on host) --
    x = nc.declare_dram_parameter("x", [C, n_loc], BF16, isOutput=False)
    r1 = nc.declare_dram_parameter("r1", [C, n_loc], BF16, isOutput=False)
    r2 = nc.declare_dram_parameter("r2", [C, n_loc], BF16, isOutput=False)
    # wkv_j: [128, 2*256] col chunk 256k = [WkT | WvT](our heads)[128k:, :]
    wkv1 = nc.declare_dram_parameter("wkv1", [128, 2 * 256], BF16, isOutput=False)
    # wall: [wkv2(512) | wq(256) | wo(512)] packed so one DMA covers them;
    # wq col chunk 128k = Wq.T[128k:, our 128 ch]; wo col chunk 256k =
    # Wo.T[our 256 concat channels][128k:, :]
    wall = nc.declare_dram_parameter("wall", [128, 1280], BF16, isOutput=False)
    # fp16 consts packed: [ones4(32) | ones4T(128)]
    cwall = nc.declare_dram_parameter("cwall", [128, 160], F16, isOutput=False)

    y = nc.declare_dram_parameter("y", [C, n_loc], BF16, isOutput=True)

    refs = [r1, r2]

    with tile.TileContext(nc) as tc:
        with (
            tc.tile_pool(name="weights", bufs=1) as wpool,
            tc.tile_pool(name="persist", bufs=1) as ppool,
            tc.tile_pool(name="io", bufs=4) as iopool,
            tc.tile_pool(name="work", bufs=3) as wkpool,
        ):
            # ---- weights / constants (wkv1 first on sync so branch-0 can
            # start; the packed walls ride the gpsimd queue) ----
            wkv1_t = wpool.tile([128, 2 * 256], BF16, tag="wkv0", name="wkv_t0")
            nc.sync.dma_start(wkv1_t[:], wkv1[:, :])
            wall_t = wpool.tile([128, 1280], BF16, tag="wall")
            nc.gpsimd.dma_start(wall_t[:], wall[:, :])
            cwall_t = wpool.tile([128, 160], F16, tag="cwall")
            nc.gpsimd.dma_start(cwall_t[:], cwall[:, :])
            wkv_t = [wkv1_t, wall_t[:, 0:512]]
            wq_t = wall_t[:, 512:768]
            wo_t = wall_t[:, 768:1280]
            ones4_t = cwall_t[:, 0:32]
            ones4T_t = cwall_t[:, 32:160]

            compact = ppool.tile([128, 64], F32, tag="compact")
            compact_bf = ppool.tile([128, 64], BF16, tag="compact_bf")
            mt_sb = ppool.tile([128, 256], F16, tag="mt_sb")
            recipT = ppool.tile([128, 2], F32, tag="recipT")
            expq = ppool.tile([128, n_loc], F16, tag="expq")
            nzc = (nt512 + 3) // 4
            zqr = ppool.tile([128, 512 * nzc], F16, tag="zqr")

            CH = 4               # kv: 128-pixel tiles per chunk
            nch = nt // CH       # 32 chunks per branch

            with (
                tc.tile_pool(name="kvstage", bufs=1) as kvpool,
                tc.tile_pool(name="psA", bufs=2, space="PSUM") as psA,
                tc.tile_pool(name="psAcc", bufs=1, space="PSUM") as psAcc,
                tc.tile_pool(name="psQ", bufs=2, space="PSUM") as psQ,
            ):
                # ek/vt live in fp8: per-pixel quantization noise averages
                # out over the 16384-pixel ctx contraction.  exp is biased
                # by -3 so ek stays inside fp8 e4m3 range; the bias cancels
                # exactly in ctx/zk.
                ekt_all = kvpool.tile([128, nt * 128], FP8, tag="ekt_all")
                vt_all = kvpool.tile([128, nt * 128], FP8, tag="vt_all")
                ones1_t = wpool.tile([128, 2], FP8, tag="o1")
                nc.vector.memset(ones1_t[:], 1.0)
                bias3 = wpool.tile([128, 1], F32, tag="bias3")
                nc.vector.memset(bias3[:], -3.0)
                # cols 128j..128j+128 = branch j ctx[c,d]
                ctx_ps = psAcc.tile([128, 256], F32, tag="ctx")
                # zkT col j = sum_n exp(k_j[c,n]) on partitions c
                zkT_ps = psAcc.tile([128, 2], F32, tag="zkT")

                def pass1(j, ch):
                    base = ch * CH * 128
                    r_t = iopool.tile([128, CH * 256], BF16, tag="rchunk",
                                      name=f"r_{j}_{ch}")
                    for k in range(2):
                        dma_eng = nc.sync if (ch + k) % 2 == 0 else nc.gpsimd
                        dma_eng.dma_start(
                            r_t[:, CH * 128 * k:CH * 128 * (k + 1)],
                            refs[j][128 * k:128 * (k + 1), base:base + CH * 128])
                    kv_ps = psA.tile([128, CH * 256], F32, tag="kv",
                                     name=f"kv_{j}_{ch}")
                    for t in range(CH):
                        for k in range(2):
                            nc.tensor.matmul(
                                kv_ps[:, 256 * t:256 * (t + 1)],
                                r_t[:, CH * 128 * k + 128 * t:
                                       CH * 128 * k + 128 * (t + 1)],
                                wkv_t[j][:, 256 * k:256 * (k + 1)],
                                start=(k == 0), stop=(k == 1),
                            )
                    ek_sl = ekt_all[:, ch * CH * 128:(ch + 1) * CH * 128]
                    nc.scalar.activation(
                        ek_sl.rearrange("p (t c) -> p t c", t=CH),
                        kv_ps[:].rearrange("p (t c) -> p t c", t=CH)[:, :, 0:128],
                        AF.Exp, bias=bias3[:],
                    )
                    vt_sl = vt_all[:, ch * CH * 128:(ch + 1) * CH * 128]
                    nc.vector.tensor_copy(
                        vt_sl.rearrange("p (t c) -> p t c", t=CH),
                        kv_ps[:].rearrange("p (t c) -> p t c", t=CH)[:, :, 128:256],
                    )

                def pass2(j, ch):
                    # fp8 DoubleRow over tile pairs; the N=1 ones-matmul
                    # reuses the same ekt stationary to accumulate zkT[c].
                    for tp in range(ch * CH // 2, (ch + 1) * CH // 2):
                        ek_pair = ekt_all[:, 256 * tp:256 * (tp + 1)].rearrange(
                            "p (k c) -> p k c", k=2)
                        nc.tensor.matmul(
                            ctx_ps[:, 128 * j:128 * (j + 1)],
                            ek_pair,
                            vt_all[:, 256 * tp:256 * (tp + 1)].rearrange(
                                "p (k c) -> p k c", k=2),
                            start=(tp == 0), stop=(tp == nt // 2 - 1),
                            perf_mode=DOUBLE_ROW,
                        )
                        nc.tensor.matmul(
                            zkT_ps[:, j:j + 1],
                            ek_pair,
                            ones1_t[:].rearrange("p (k c) -> p k c", k=2),
                            start=(tp == 0), stop=(tp == nt // 2 - 1),
                            perf_mode=DOUBLE_ROW,
                        )

                def compact_j(j):
                    # fold 1/zk[c] into the diag 32x32 blocks of ctx[c,d]
                    # (c on partitions), cast to bf16
                    nc.vector.reciprocal_approx_fast(
                        recipT[:, j:j + 1], zkT_ps[:, j:j + 1])
                    for a in range(4):
                        nc.vector.tensor_scalar_mul(
                            compact[32 * a:32 * (a + 1), 32 * j:32 * j + 32],
                            ctx_ps[32 * a:32 * (a + 1),
                                   128 * j + 32 * a:128 * j + 32 * (a + 1)],
                            recipT[32 * a:32 * (a + 1), j:j + 1],
                        )
                    nc.vector.tensor_copy(
                        compact_bf[:, 32 * j:32 * j + 32],
                        compact[:, 32 * j:32 * j + 32])

                def qchunk(i):
                    base = i * 512
                    x_t = iopool.tile([128, 1024], BF16, tag="xchunk",
                                      name=f"x_{i}")
                    for k in range(2):
                        dma_eng = nc.sync if (i + k) % 2 == 0 else nc.gpsimd
                        dma_eng.dma_start(
                            x_t[:, 512 * k:512 * (k + 1)],
                            x[128 * k:128 * (k + 1), base:base + 512])
                    q_ps = psQ.tile([128, 512], F32, tag="q", name=f"q_{i}")
                    for k in range(2):
                        nc.tensor.matmul(
                            q_ps[:], wq_t[:, 128 * k:128 * (k + 1)],
                            x_t[:, 512 * k:512 * (k + 1)],
                            start=(k == 0), stop=(k == 1),
                        )
                    nc.scalar.activation(
                        expq[:, base:base + 512], q_ps[:], AF.Exp)

                def zqchunk(tc4):
                    zq_ps = psQ.tile([128, 512], F32, tag="q", name=f"zq_{tc4}")
                    for u in range(4):
                        t = 4 * tc4 + u
                        nc.tensor.matmul(
                            zq_ps[32 * u:32 * u + 32, :], ones4_t[:],
                            expq[:, 512 * t:512 * (t + 1)],
                            start=True, stop=True,
                            tile_position=(0, 32 * u),
                        )
                    zq_f = wkpool.tile([128, 512], F32, tag="zq_f",
                                       name=f"zqf_{tc4}")
                    nc.vector.reciprocal_approx_fast(zq_f[:], zq_ps[:])
                    nc.vector.tensor_copy(
                        zqr[:, 512 * tc4:512 * (tc4 + 1)], zq_f[:])

                # ---- branch 0: kv chunks with trailing ctx/zk batches ----
                for ch in range(nch):
                    pass1(0, ch)
                    if ch > 0:
                        pass2(0, ch - 1)
                pass2(0, nch - 1)
                compact_j(0)

                # ---- branch 1 interleaved with q projection + zq ----
                for ch in range(nch):
                    pass1(1, ch)
                    if ch > 0:
                        pass2(1, ch - 1)
                    qchunk(ch)
                    if ch % 4 == 3:
                        zqchunk(ch // 4)
                        for t in range(4 * (ch // 4), 4 * (ch // 4) + 4):
                            u, tc4 = t % 4, t // 4
                            zqb_ps = psQ.tile([128, 512], F32, tag="q",
                                              name=f"zqb_{t}")
                            # K=32 selector over the 8x-replicated recip rows
                            # streams faster than the K=4 broadcast
                            nc.tensor.matmul(
                                zqb_ps[:], ones4T_t[32 * u:32 * (u + 1), :],
                                zqr[32 * u:32 * (u + 1),
                                    512 * tc4:512 * (tc4 + 1)],
                                start=True, stop=True,
                                tile_position=(32 * u, 0),
                            )
                            nc.vector.tensor_mul(
                                expq[:, 512 * t:512 * (t + 1)],
                                expq[:, 512 * t:512 * (t + 1)],
                                zqb_ps[:],
                            )
                pass2(1, nch - 1)
                compact_j(1)

            # MT[d, o] = sum_j sum_c ctx_j[c,d]/zk_j[c] * WoT_j[c, o];
            # head-block-diagonal, so 4 PE-packed matmuls per branch.  Kept
            # as f32r so the y matmul loses no precision on the weights.
            with tc.tile_pool(name="psM", bufs=1, space="PSUM") as psM:
                mt_ps = psM.tile([128, 256], F32, tag="mt")
                for j in range(2):
                    for a in range(4):
                        nc.tensor.matmul(
                            mt_ps[32 * a:32 * (a + 1), :],
                            compact_bf[32 * a:32 * (a + 1),
                                       32 * j:32 * j + 32],
                            wo_t[32 * a:32 * (a + 1), 256 * j:256 * (j + 1)],
                            start=(j == 0), stop=(j == 1),
                            tile_position=(32 * a, 32 * a),
                        )
                nc.vector.tensor_copy(mt_sb[:], mt_ps[:])

            # ======= Phase C: y tile = MT^T @ q-hat, 2 matmuls per tile ====
            # Tiles processed in pairs so each half-row y DMA moves 2 KB
            # per partition (half the dispatch count).
            with (
                tc.tile_pool(name="psC", bufs=6, space="PSUM") as psC,
                tc.tile_pool(name="ysb", bufs=4) as ypool,
            ):
                for g in range(nt512 // 2):
                    y_sb = ypool.tile([128, 2048], BF16, tag="ysb",
                                      name=f"ysb_{g}")
                    for u in range(2):
                        t = 2 * g + u
                        for m in range(2):
                            y_ps = psC.tile([128, 512], F32, tag="y_ps",
                                            name=f"yps_{t}_{m}", bufs=6)
                            nc.tensor.matmul(
                                y_ps[:], mt_sb[:, 128 * m:128 * (m + 1)],
                                expq[:, 512 * t:512 * (t + 1)],
                                start=True, stop=True,
                            )
                            csl = y_sb[:, 1024 * m + 512 * u:
                                          1024 * m + 512 * (u + 1)]
                            if m == 0:
                                nc.vector.tensor_copy(csl, y_ps[:])
                            else:
                                nc.scalar.copy(csl, y_ps[:])
                    for m in range(2):
                        dma_eng = nc.sync if (g + m) % 2 == 0 else nc.gpsimd
                        dma_eng.dma_start(
                            y[128 * m:128 * (m + 1), 1024 * g:1024 * (g + 1)],
                            y_sb[:, 1024 * m:1024 * (m + 1)])

    nc.compile()
    return nc


def _consts():
    ones4 = np.zeros((128, 32), dtype=np.float16)
    for col in range(32):
        a = col % 4
        ones4[32 * a:32 * (a + 1), col] = 1
    # selector: row 32u + a + 4*(d%8) feeds output channel 32a + d, picking
    # one of the 8 identical per-head sum replicas per output row
    ones4T = np.zeros((128, 128), dtype=np.float16)
    for u in range(4):
        for a in range(4):
            for d in range(32):
                ones4T[32 * u + a + 4 * (d % 8), 32 * a + d] = 1
    return ones4, ones4T


def _ktile(wT):
    """[C_in, C_out] -> [128, (C_in//128)*C_out] k-tiles along the free dim."""
    kin = wT.shape[0] // 128
    return np.concatenate([wT[128 * k:128 * (k + 1), :] for k in range(kin)], axis=1)


def make_in_maps(x, ref_1, ref_2, Wq, Wk1, Wk2, Wv1, Wv2, Wo, n_loc=N):
    bf = ml_dtypes.bfloat16
    ones4, ones4T = _consts()
    xf = np.asarray(x).reshape(B, C, -1)
    r1f = np.asarray(ref_1).reshape(B, C, -1)
    r2f = np.asarray(ref_2).reshape(B, C, -1)
    WqT, WoT = np.asarray(Wq).T, np.asarray(Wo).T
    WkT = [np.asarray(Wk1).T, np.asarray(Wk2).T]
    WvT = [np.asarray(Wv1).T, np.asarray(Wv2).T]
    gw = {}
    for g in range(2):
        sl = slice(128 * g, 128 * (g + 1))
        wq_g = np.ascontiguousarray(_ktile(WqT[:, sl])).astype(bf)
        wkv_g = [np.ascontiguousarray(
            _ktile(np.concatenate([WkT[j][:, sl], WvT[j][:, sl]], axis=1))
        ).astype(bf) for j in range(2)]
        # Wo rows for our concat channels: branch1 128g.., branch2 256+128g..
        wo_rows = np.concatenate(
            [WoT[sl, :], WoT[256 + 128 * g:256 + 128 * (g + 1), :]], axis=0)
        wo_g = np.ascontiguousarray(_ktile(wo_rows)).astype(bf)
        gw[g] = (wq_g, wkv_g[0], wkv_g[1], wo_g)
    cwall = np.ascontiguousarray(
        np.concatenate([ones4, ones4T], axis=1))
    in_maps = []
    for core in range(N_CORES):
        b, g = core // 2, core % 2
        wq_g, wkv1_g, wkv2_g, wo_g = gw[g]
        wall = np.ascontiguousarray(
            np.concatenate([wkv2_g, wq_g, wo_g], axis=1))
        in_maps.append({
            "x": np.ascontiguousarray(xf[b, :, :n_loc]).astype(bf),
            "r1": np.ascontiguousarray(r1f[b, :, :n_loc]).astype(bf),
            "r2": np.ascontiguousarray(r2f[b, :, :n_loc]).astype(bf),
            "wkv1": wkv1_g, "wall": wall, "cwall": cwall,
        })
    return in_maps


_NC_CACHE = {}


def kernel(x, ref_1, ref_2, Wq, Wk1, Wk2, Wv1, Wv2, Wo, _trace=False):
    n_loc = N
    if n_loc not in _NC_CACHE:
        _NC_CACHE[n_loc] = build_nc(n_loc)
    nc = _NC_CACHE[n_loc]
    in_maps = make_in_maps(x, ref_1, ref_2, Wq, Wk1, Wk2, Wv1, Wv2, Wo, n_loc)
    res = run_bass_kernel_spmd(nc, in_maps, core_ids=list(range(N_CORES)),
                               trace=_trace)
    out = np.empty((B, C, n_loc), dtype=np.float32)
    for b in range(B):
        out[b] = (res.results[2 * b]["y"].astype(np.float32)
                  + res.results[2 * b + 1]["y"].astype(np.float32))
    if _trace:
        kernel.last_results = res
    return out.reshape(B, C, H, W)



# revision 80
# speedup vs baseline: 1.3348x; 1.0557x over previous
"""Bidirectional cross-attention kernel for Trainium2, SPMD over 8 NeuronCores.

Reference (per batch b, heads K=8, head dim D=32, N=128*128 pixels):
    q   = softmax_d(Wq @ x)
    for branch j in {1,2}:
        key   = softmax_n(Wk_j @ ref_j)          # softmax over the pixel dim
        v     = Wv_j @ ref_j
        ctx_j = key @ v^T                        # [K,D,D]
        out_j = per-pixel  q @ ctx_j^T
    y = Wo @ concat(out_1, out_2)

Sharding: 8 cores = batch(4) x head-group(2).  Each core owns 4 of the 8
heads for its batch: projections, softmaxes, ctx and the out einsum are
fully head-local; the final Wo projection is computed as a partial sum
over the core's 256 (of 512) concat channels, and the host adds the two
partial outputs per batch.  No cross-core communication on device.

Key algebraic fusion: both branches share the same query, so
    y = Wo @ concat(C1 q, C2 q) = (Wo1 C1 + Wo2 C2) q = M q
where C_j = ctx_j / zk_j is tiny ([d,d] per head, block-diagonal).  M is
built once per core from the PSUM-resident ctx blocks (with 1/zk folded
in while the key channel is still on partitions), and the entire output
phase is just 2 matmuls of M^T against each 512-pixel q-hat tile.  zk
rides the ctx accumulation as an N=1 ones-matmul per tile pair that
reuses the same ekt stationary.

Numerics: bf16 projection inputs, fp8(e4m3) ek/vt for the ctx DoubleRow
matmuls (per-pixel quantization noise averages out over the 16384-pixel
contraction; exp is biased by -3 to stay in fp8 range, which cancels in
ctx/zk), fp16 for the whole q pipeline and M (10-bit mantissa keeps the
fused-weight rounding well under the error budget), fp32 PSUM
accumulation.  Softmaxes skip max-subtraction (logits ~N(0,1)).

SBUF layout: tensors with >128 channels are stored as [128, k*cols] with
128-channel k-tiles side by side in the free dim.  Key/value tensors are
kept in transposed [pixel, channel] layout (needed for the ctx matmul,
whose contraction runs over pixels).
"""

import numpy as np
import ml_dtypes

import concourse.bass as bass
import concourse.bacc as bacc
import concourse.tile as tile
from concourse import mybir
from concourse.bass_utils import run_bass_kernel_spmd

BF16 = mybir.dt.bfloat16
F16 = mybir.dt.float16
F32 = mybir.dt.float32
FP8 = mybir.dt.float8e4
DOUBLE_ROW = mybir.MatmulPerfMode.DoubleRow
AF = mybir.ActivationFunctionType

B, C, H, W = 4, 256, 128, 128
K, D = 8, 32
N = H * W
N_CORES = 8


def build_nc(n_loc=N):
    nc = bacc.Bacc("TRN2", target_bir_lowering=False, debug=False,
                   num_devices=N_CORES)

    nt = n_loc // 128        # 128-pixel tiles (128)
    nt512 = n_loc // 512     # 512-pixel tiles (32)

    # ---- I/O (weights pre-transposed, head-group-sliced, k-tiled on host) --
    x = nc.declare_dram_parameter("x", [C, n_loc], BF16, isOutput=False)
    r1 = nc.declare_dram_parameter("r1", [C, n_loc], BF16, isOutput=False)
    r2 = nc.declare_dram_parameter("r2", [C, n_loc], BF16, isOutput=False)
    # wkv_j: [128, 2*256] col chunk 256k = [WkT | WvT](our heads)[128k:, :]
    wkv1 = nc.declare_dram_parameter("wkv1", [128, 2 * 256], BF16, isOutput=False)
    # wall: [wkv2(512) | wq(256) | wo(512)] packed so one DMA covers them;
    # wq col chunk 128k = Wq.T[128k:, our 128 ch]; wo col chunk 256k =
    # Wo.T[our 256 concat channels][128k:, :]
    wall = nc.declare_dram_parameter("wall", [128, 1280], BF16, isOutput=False)
    # fp16 consts packed: [ones4(32) | ones4T(128)]
    cwall = nc.declare_dram_parameter("cwall", [128, 160], F16, isOutput=False)

    y = nc.declare_dram_parameter("y", [C, n_loc], BF16, isOutput=True)

    refs = [r1, r2]

    with tile.TileContext(nc) as tc:
        with (
            tc.tile_pool(name="weights", bufs=1) as wpool,
            tc.tile_pool(name="persist", bufs=1) as ppool,
            tc.tile_pool(name="io", bufs=4) as iopool,
            tc.tile_pool(name="work", bufs=3) as wkpool,
        ):
            # ---- weights / constants (wkv1 first on sync so branch-0 can
            # start; the packed walls ride the gpsimd queue) ----
            wkv1_t = wpool.tile([128, 2 * 256], BF16, tag="wkv0", name="wkv_t0")
            nc.sync.dma_start(wkv1_t[:], wkv1[:, :])
            wall_t = wpool.tile([128, 1280], BF16, tag="wall")
            nc.gpsimd.dma_start(wall_t[:], wall[:, :])
            cwall_t = wpool.tile([128, 160], F16, tag="cwall")
            nc.gpsimd.dma_start(cwall_t[:], cwall[:, :])
            wkv_t = [wkv1_t, wall_t[:, 0:512]]
            wq_t = wall_t[:, 512:768]
            wo_t = wall_t[:, 768:1280]
            ones4_t = cwall_t[:, 0:32]
            ones4T_t = cwall_t[:, 32:160]

            compact = ppool.tile([128, 64], F32, tag="compact")
            compact_bf = ppool.tile([128, 64], BF16, tag="compact_bf")
            mt_sb = ppool.tile([128, 256], F16, tag="mt_sb")
            recipT = ppool.tile([128, 2], F32, tag="recipT")
            expq = ppool.tile([128, n_loc], F16, tag="expq")
            nzc = (nt512 + 3) // 4
            zqr = ppool.tile([128, 512 * nzc], F16, tag="zqr")

            CH = 4               # kv: 128-pixel tiles per chunk
            nch = nt // CH       # 32 chunks per branch

            with (
                tc.tile_pool(name="kvstage", bufs=1) as kvpool,
                tc.tile_pool(name="psA", bufs=2, space="PSUM") as psA,
                tc.tile_pool(name="psAcc", bufs=1, space="PSUM") as psAcc,
                tc.tile_pool(name="psQ", bufs=2, space="PSUM") as psQ,
            ):
                # ek/vt live in fp8: per-pixel quantization noise averages
                # out over the 16384-pixel ctx contraction.  exp is biased
                # by -3 so ek stays inside fp8 e4m3 range; the bias cancels
                # exactly in ctx/zk.
                ekt_all = kvpool.tile([128, nt * 128], FP8, tag="ekt_all")
                vt_all = kvpool.tile([128, nt * 128], FP8, tag="vt_all")
                ones1_t = wpool.tile([128, 2], FP8, tag="o1")
                nc.vector.memset(ones1_t[:], 1.0)
                bias3 = wpool.tile([128, 1], F32, tag="bias3")
                nc.vector.memset(bias3[:], -3.0)
                # cols 128j..128j+128 = branch j ctx[c,d]
                ctx_ps = psAcc.tile([128, 256], F32, tag="ctx")
                # zkT col j = sum_n exp(k_j[c,n]) on partitions c
                zkT_ps = psAcc.tile([128, 2], F32, tag="zkT")

                def pass1(j, ch):
                    base = ch * CH * 128
                    r_t = iopool.tile([128, CH * 256], BF16, tag="rchunk",
                                      name=f"r_{j}_{ch}")
                    for k in range(2):
                        dma_eng = nc.sync if (ch + k) % 2 == 0 else nc.gpsimd
                        dma_eng.dma_start(
                            r_t[:, CH * 128 * k:CH * 128 * (k + 1)],
                            refs[j][128 * k:128 * (k + 1), base:base + CH * 128])
                    kv_ps = psA.tile([128, CH * 256], F32, tag="kv",
                                     name=f"kv_{j}_{ch}")
                    for t in range(CH):
                        for k in range(2):
                            nc.tensor.matmul(
                                kv_ps[:, 256 * t:256 * (t + 1)],
                                r_t[:, CH * 128 * k + 128 * t:
                                       CH * 128 * k + 128 * (t + 1)],
                                wkv_t[j][:, 256 * k:256 * (k + 1)],
                                start=(k == 0), stop=(k == 1),
                            )
                    ek_sl = ekt_all[:, ch * CH * 128:(ch + 1) * CH * 128]
                    nc.scalar.activation(
                        ek_sl.rearrange("p (t c) -> p t c", t=CH),
                        kv_ps[:].rearrange("p (t c) -> p t c", t=CH)[:, :, 0:128],
                        AF.Exp, bias=bias3[:],
                    )
                    vt_sl = vt_all[:, ch * CH * 128:(ch + 1) * CH * 128]
                    nc.vector.tensor_copy(
                        vt_sl.rearrange("p (t c) -> p t c", t=CH),
                        kv_ps[:].rearrange("p (t c) -> p t c", t=CH)[:, :, 128:256],
                    )

                def pass2(j, ch):
                    # fp8 DoubleRow over tile pairs; the N=1 ones-matmul
                    # reuses the same ekt stationary to accumulate zkT[c].
                    for tp in range(ch * CH // 2, (ch + 1) * CH // 2):
                        ek_pair = ekt_all[:, 256 * tp:256 * (tp + 1)].rearrange(
                            "p (k c) -> p k c", k=2)
                        nc.tensor.matmul(
                            ctx_ps[:, 128 * j:128 * (j + 1)],
                            ek_pair,
                            vt_all[:, 256 * tp:256 * (tp + 1)].rearrange(
                                "p (k c) -> p k c", k=2),
                            start=(tp == 0), stop=(tp == nt // 2 - 1),
                            perf_mode=DOUBLE_ROW,
                        )
                        nc.tensor.matmul(
                            zkT_ps[:, j:j + 1],
                            ek_pair,
                            ones1_t[:].rearrange("p (k c) -> p k c", k=2),
                            start=(tp == 0), stop=(tp == nt // 2 - 1),
                            perf_mode=DOUBLE_ROW,
                        )

                def compact_j(j):
                    # fold 1/zk[c] into the diag 32x32 blocks of ctx[c,d]
                    # (c on partitions), cast to bf16
                    nc.vector.reciprocal_approx_fast(
                        recipT[:, j:j + 1], zkT_ps[:, j:j + 1])
                    for a in range(4):
                        nc.vector.tensor_scalar_mul(
                            compact[32 * a:32 * (a + 1), 32 * j:32 * j + 32],
                            ctx_ps[32 * a:32 * (a + 1),
                                   128 * j + 32 * a:128 * j + 32 * (a + 1)],
                            recipT[32 * a:32 * (a + 1), j:j + 1],
                        )
                    nc.vector.tensor_copy(
                        compact_bf[:, 32 * j:32 * j + 32],
                        compact[:, 32 * j:32 * j + 32])

                def qchunk(i):
                    base = i * 512
                    x_t = iopool.tile([128, 1024], BF16, tag="xchunk",
                                      name=f"x_{i}")
                    for k in range(2):
                        dma_eng = nc.sync if (i + k) % 2 == 0 else nc.gpsimd
                        dma_eng.dma_start(
                            x_t[:, 512 * k:512 * (k + 1)],
                            x[128 * k:128 * (k + 1), base:base + 512])
                    q_ps = psQ.tile([128, 512], F32, tag="q", name=f"q_{i}")
                    for k in range(2):
                        nc.tensor.matmul(
                            q_ps[:], wq_t[:, 128 * k:128 * (k + 1)],
                            x_t[:, 512 * k:512 * (k + 1)],
                            start=(k == 0), stop=(k == 1),
                        )
                    nc.scalar.activation(
                        expq[:, base:base + 512], q_ps[:], AF.Exp)

                def zqchunk(tc4):
                    zq_ps = psQ.tile([128, 512], F32, tag="q", name=f"zq_{tc4}")
                    for u in range(4):
                        t = 4 * tc4 + u
                        nc.tensor.matmul(
                            zq_ps[32 * u:32 * u + 32, :], ones4_t[:],
                            expq[:, 512 * t:512 * (t + 1)],
                            start=True, stop=True,
                            tile_position=(0, 32 * u),
                        )
                    zq_f = wkpool.tile([128, 512], F32, tag="zq_f",
                                       name=f"zqf_{tc4}")
                    nc.vector.reciprocal_approx_fast(zq_f[:], zq_ps[:])
                    nc.vector.tensor_copy(
                        zqr[:, 512 * tc4:512 * (tc4 + 1)], zq_f[:])

                # ---- branch 0: kv chunks with trailing ctx/zk batches ----
                for ch in range(nch):
                    pass1(0, ch)
                    if ch > 0:
                        pass2(0, ch - 1)
                pass2(0, nch - 1)
                compact_j(0)

                # ---- branch 1 interleaved with q projection + zq ----
                for ch in range(nch):
                    pass1(1, ch)
                    if ch > 0:
                        pass2(1, ch - 1)
                    qchunk(ch)
                    if ch % 4 == 3:
                        zqchunk(ch // 4)
                        for t in range(4 * (ch // 4), 4 * (ch // 4) + 4):
                            u, tc4 = t % 4, t // 4
                            zqb_ps = psQ.tile([128, 512], F32, tag="q",
                                              name=f"zqb_{t}")
                            # K=32 selector over the 8x-replicated recip rows
                            # streams faster than the K=4 broadcast
                            nc.tensor.matmul(
                                zqb_ps[:], ones4T_t[32 * u:32 * (u + 1), :],
                                zqr[32 * u:32 * (u + 1),
                                    512 * tc4:512 * (tc4 + 1)],
                                start=True, stop=True,
                                tile_position=(32 * u, 0),
                            )
                            nc.vector.tensor_mul(
                                expq[:, 512 * t:512 * (t + 1)],
                                expq[:, 512 * t:512 * (t + 1)],
                                zqb_ps[:],
                            )
                pass2(1, nch - 1)
                compact_j(1)

            # MT[d, o] = sum_j sum_c ctx_j[c,d]/zk_j[c] * WoT_j[c, o];
            # head-block-diagonal, so 4 PE-packed matmuls per branch.  Kept
            # as f32r so the y matmul loses no precision on the weights.
            with tc.tile_pool(name="psM", bufs=1, space="PSUM") as psM:
                mt_ps = psM.tile([128, 256], F32, tag="mt")
                for j in range(2):
                    for a in range(4):
                        nc.tensor.matmul(
                            mt_ps[32 * a:32 * (a + 1), :],
                            compact_bf[32 * a:32 * (a + 1),
                                       32 * j:32 * j + 32],
                            wo_t[32 * a:32 * (a + 1), 256 * j:256 * (j + 1)],
                            start=(j == 0), stop=(j == 1),
                            tile_position=(32 * a, 32 * a),
                        )
                nc.vector.tensor_copy(mt_sb[:], mt_ps[:])

            # ======= Phase C: y tile = MT^T @ q-hat, 2 matmuls per tile ====
            # Tiles processed in pairs so each half-row y DMA moves 2 KB
            # per partition (half the dispatch count).
            with (
                tc.tile_pool(name="psC", bufs=6, space="PSUM") as psC,
                tc.tile_pool(name="ysb", bufs=4) as ypool,
            ):
                for g in range(nt512 // 2):
                    y_sb = ypool.tile([128, 2048], BF16, tag="ysb",
                                      name=f"ysb_{g}")
                    for u in range(2):
                        t = 2 * g + u
                        for m in range(2):
                            y_ps = psC.tile([128, 512], F32, tag="y_ps",
                                            name=f"yps_{t}_{m}", bufs=6)
                            nc.tensor.matmul(
                                y_ps[:], mt_sb[:, 128 * m:128 * (m + 1)],
                                expq[:, 512 * t:512 * (t + 1)],
                                start=True, stop=True,
                            )
                            csl = y_sb[:, 1024 * m + 512 * u:
                                          1024 * m + 512 * (u + 1)]
                            if m == 0:
                                nc.vector.tensor_copy(csl, y_ps[:])
                            else:
                                nc.scalar.copy(csl, y_ps[:])
                    for m in range(2):
                        dma_eng = nc.sync if (g + m) % 2 == 0 else nc.gpsimd
                        dma_eng.dma_start(
                            y[128 * m:128 * (m + 1), 1024 * g:1024 * (g + 1)],
                            y_sb[:, 1024 * m:1024 * (m + 1)])

    nc.compile()
    return nc


def _consts():
    ones4 = np.zeros((128, 32), dtype=np.float16)
    for col in range(32):
        a = col % 4
        ones4[32 * a:32 * (a + 1), col] = 1
    # selector: row 32u + a + 4*(d%8) feeds output channel 32a + d, picking
    # one of the 8 identical per-head sum replicas per output row
    ones4T = np.zeros((128, 128), dtype=np.float16)
    for u in range(4):
        for a in range(4):
            for d in range(32):
                ones4T[32 * u + a + 4 * (d % 8), 32 * a + d] = 1
    return ones4, ones4T


def _ktile(wT):
    """[C_in, C_out] -> [128, (C_in//128)*C_out] k-tiles along the free dim."""
    kin = wT.shape[0] // 128
    return np.concatenate([wT[128 * k:128 * (k + 1), :] for k in range(kin)], axis=1)


def make_in_maps(x, ref_1, ref_2, Wq, Wk1, Wk2, Wv1, Wv2, Wo, n_loc=N):
    bf = ml_dtypes.bfloat16
    ones4, ones4T = _consts()
    xf = np.asarray(x).reshape(B, C, -1)
    r1f = np.asarray(ref_1).reshape(B, C, -1)
    r2f = np.asarray(ref_2).reshape(B, C, -1)
    WqT, WoT = np.asarray(Wq).T, np.asarray(Wo).T
    WkT = [np.asarray(Wk1).T, np.asarray(Wk2).T]
    WvT = [np.asarray(Wv1).T, np.asarray(Wv2).T]
    gw = {}
    for g in range(2):
        sl = slice(128 * g, 128 * (g + 1))
        wq_g = np.ascontiguousarray(_ktile(WqT[:, sl])).astype(bf)
        wkv_g = [np.ascontiguousarray(
            _ktile(np.concatenate([WkT[j][:, sl], WvT[j][:, sl]], axis=1))
        ).astype(bf) for j in range(2)]
        # Wo rows for our concat channels: branch1 128g.., branch2 256+128g..
        wo_rows = np.concatenate(
            [WoT[sl, :], WoT[256 + 128 * g:256 + 128 * (g + 1), :]], axis=0)
        wo_g = np.ascontiguousarray(_ktile(wo_rows)).astype(bf)
        gw[g] = (wq_g, wkv_g[0], wkv_g[1], wo_g)
    cwall = np.ascontiguousarray(
        np.concatenate([ones4, ones4T], axis=1))
    in_maps = []
    for core in range(N_CORES):
        b, g = core // 2, core % 2
        wq_g, wkv1_g, wkv2_g, wo_g = gw[g]
        wall = np.ascontiguousarray(
            np.concatenate([wkv2_g, wq_g, wo_g], axis=1))
        in_maps.append({
            "x": np.ascontiguousarray(xf[b, :, :n_loc]).astype(bf),
            "r1": np.ascontiguousarray(r1f[b, :, :n_loc]).astype(bf),
            "r2": np.ascontiguousarray(r2f[b, :, :n_loc]).astype(bf),
            "wkv1": wkv1_g, "wall": wall, "cwall": cwall,
        })
    return in_maps


_NC_CACHE = {}


def kernel(x, ref_1, ref_2, Wq, Wk1, Wk2, Wv1, Wv2, Wo, _trace=False):
    n_loc = N
    if n_loc not in _NC_CACHE:
        _NC_CACHE[n_loc] = build_nc(n_loc)
    nc = _NC_CACHE[n_loc]
    in_maps = make_in_maps(x, ref_1, ref_2, Wq, Wk1, Wk2, Wv1, Wv2, Wo, n_loc)
    res = run_bass_kernel_spmd(nc, in_maps, core_ids=list(range(N_CORES)),
                               trace=_trace)
    out = np.empty((B, C, n_loc), dtype=np.float32)
    for b in range(B):
        out[b] = (res.results[2 * b]["y"].astype(np.float32)
                  + res.results[2 * b + 1]["y"].astype(np.float32))
    if _trace:
        kernel.last_results = res
    return out.reshape(B, C, H, W)

